# revision 19
# baseline (speedup 1.0000x reference)
"""CSWM transition GNN kernel for 8 TRN2 NeuronCores.

Sharding: data-parallel over the 512 edge-groups (the quirky edge list is
block-diagonal over groups of 15 consecutive flat rows). Each core gets
64 groups (960 edge rows) + 64 of the 512 zero-agg tail rows = 1024 node
rows. No cross-core communication.

Host-side algebra:
  - cat(xi,xi,xj)@e_w0 = xi@(W0a+W0b) + xj@W0c          (per-node U,V)
  - final edge matmul commutes with scatter-add; W2 then folds into the
    node MLP first layer: nw0s = e_w2 @ n_w0[532:1556]
  - per-edge work: one 1024x1024 matmul + LayerNorm + relu
"""

import numpy as np
import ml_dtypes

import concourse.bass as bass
import concourse.mybir as mybir
import concourse.tile as tile
from concourse import bacc
from concourse.bass_utils import run_bass_kernel_spmd
from concourse.masks import make_identity

BF16 = mybir.dt.bfloat16
F32 = mybir.dt.float32
F8 = mybir.dt.float8e4
DR = mybir.MatmulPerfMode.DoubleRow
AF = mybir.ActivationFunctionType

P = 128
D = 512            # embedding dim
H = 1024           # hidden dim
A_DIM = 20         # action dim
B = 512            # batch
K = 16             # objects
NG = 512           # total edge groups (block-diag over 15-row groups)
N_CORES = 8
G_CORE = NG // N_CORES          # 64 groups per core
EDGE_ROWS = G_CORE * 15         # 960
EXTRA_ROWS = (B * K - NG * 15) // N_CORES   # 64 zero-agg tail rows per core
N_ROWS = EDGE_ROWS + EXTRA_ROWS  # 1024 node rows per core
GB = 8                          # groups per aggregation block
NBLK = G_CORE // GB             # 8 blocks per core
E_BLK = GB * 225                # 1800 edges per block (incl. diagonal)
NCHUNK = (E_BLK + P - 1) // P   # 15 chunks of 128 edge-slots
NODES_BLK = GB * 15             # 120
E_PAD = 1808                    # E_BLK padded so fp8 DoubleRow k-pair stride %16==0
EPS = 1e-5
FP8 = True                      # fp8e4m3 DoubleRow for the edge matmul + aggregation


def _bf16(x):
    return np.ascontiguousarray(np.asarray(x, dtype=np.float32).astype(ml_dtypes.bfloat16))


def _f8(x):
    return np.ascontiguousarray(np.asarray(x, dtype=np.float32).astype(ml_dtypes.float8_e4m3))


def _f32(x):
    return np.ascontiguousarray(np.asarray(x, dtype=np.float32))


def _build_amat():
    """[NCHUNK*128, 120] 0/1 matrix: edge (gb,i,j) -> node gb*15+i, diagonal
    (j==i) excluded, padding rows zero."""
    a = np.zeros((NCHUNK * P, NODES_BLK), dtype=np.float32)
    for gb in range(GB):
        for i in range(15):
            for j in range(15):
                if i != j:
                    a[gb * 225 + i * 15 + j, gb * 15 + i] = 1.0
    return a


def _build_program(trivial_affine_e: bool, trivial_affine_n: bool):
    nc = bacc.Bacc("TRN2", target_bir_lowering=False, debug=False)

    # ---- DRAM parameters (per-core shards / replicated weights) ----
    def din(name, shape, dt):
        return nc.declare_dram_parameter(name, list(shape), dt, isOutput=False)

    xT = din("xT", (4, P, N_ROWS), BF16)       # x transposed, [ks,p,rows]
    actT = din("actT", (A_DIM + 1, N_ROWS), BF16)   # one-hot actions + edge-row indicator
    wab = din("wab", (4, P, H), BF16)          # W0a+W0b  [ks,p,out]
    w0c = din("w0c", (4, P, H), BF16)
    b0 = din("b0", (H,), F32)
    if FP8:
        w1 = din("w1", (8, P, H), F8)
        b1 = din("b1", (1, H), F8)
        amat = din("amat", (NCHUNK, P, P), F8)
    else:
        w1 = din("w1", (8, P, H), BF16)
        b1 = din("b1", (H,), F32)
        amat = din("amat", (NCHUNK, P, NODES_BLK), BF16)
    nw0x = din("nw0x", (4, P, H), BF16)
    nw0a = din("nw0a", (A_DIM + 1, H), BF16)   # rows 0..19 action, row 20 = e_b2 @ n_w0s
    nw0s = din("nw0s", (8, P, H), BF16)
    nb0 = din("nb0", (H,), F32)
    nw1 = din("nw1", (8, P, H), BF16)
    nb1 = din("nb1", (H,), F32)
    nw2 = din("nw2", (8, P, D), BF16)
    nb2 = din("nb2", (1, D), BF16)
    if not trivial_affine_e:
        e_g = din("e_g", (H,), F32)
        e_be = din("e_be", (H,), F32)
    if not trivial_affine_n:
        n_g = din("n_g", (H,), F32)
        n_be = din("n_be", (H,), F32)

    out = nc.declare_dram_parameter("out", [N_ROWS, D], F32, isOutput=True)

    with tile.TileContext(nc) as tc:
        with tc.tile_pool(name="const", bufs=1) as cpool:
            xT_s = cpool.tile([P, 4, N_ROWS], BF16)
            for ks in range(4):
                nc.sync.dma_start(xT_s[:, ks, :], xT[ks])
            actT_s = cpool.tile([A_DIM + 1, N_ROWS], BF16)
            nc.sync.dma_start(actT_s[:], actT[:])
            ident = cpool.tile([P, P], BF16)
            make_identity(nc, ident)
            ones_row = cpool.tile([1, P], BF16)
            nc.vector.memset(ones_row[:], 1.0)
            eps_t = cpool.tile([P, 1], F32)
            nc.vector.memset(eps_t[:], EPS)
            # sT: aggregated-hidden, transposed [feat, rows]; tail rows zero
            sT = cpool.tile([P, 8, N_ROWS], BF16)
            nc.vector.memset(sT[:, :, EDGE_ROWS:N_ROWS], 0.0)

            # ================= EDGE PHASE =================
            with (
                tc.tile_pool(name="ew", bufs=1) as ew,
                tc.tile_pool(name="uv", bufs=1) as uvp,
                tc.tile_pool(name="rp", bufs=2) as rp,
                tc.tile_pool(name="zp", bufs=6) as zp,
                tc.tile_pool(name="st", bufs=2) as stp,
                tc.tile_pool(name="ps", bufs=3 if FP8 else 2, space="PSUM") as ps,
                tc.tile_pool(name="pa", bufs=1 if FP8 else 2, space="PSUM") as pa,
            ):
                wab_s = ew.tile([P, 4, H], BF16)
                nc.sync.dma_start(wab_s[:], wab[:].rearrange("k p n -> p k n"))
                w0c_s = ew.tile([P, 4, H], BF16)
                nc.sync.dma_start(w0c_s[:], w0c[:].rearrange("k p n -> p k n"))
                b0_t = ew.tile([P, 8], F32)
                nc.sync.dma_start(b0_t[:], b0[:].rearrange("(o p) -> p o", p=P))
                if FP8:
                    w1_s = ew.tile([P, 8, H], F8)
                    nc.gpsimd.dma_start(w1_s[:], w1[:].rearrange("k p n -> p k n"))
                    amat_s = ew.tile([P, NCHUNK, P], F8)
                    nc.gpsimd.dma_start(amat_s[:], amat[:].rearrange("c p n -> p c n"))
                    b1_r = ew.tile([1, H], F8)
                    nc.sync.dma_start(b1_r[:], b1[:])
                    ones8 = ew.tile([1, P], F8)
                    nc.vector.memset(ones8[:], 1.0)
                else:
                    w1_s = ew.tile([P, 8, H], BF16)
                    nc.sync.dma_start(w1_s[:], w1[:].rearrange("k p n -> p k n"))
                    amat_s = ew.tile([P, NCHUNK, NODES_BLK], BF16)
                    nc.sync.dma_start(amat_s[:], amat[:].rearrange("c p n -> p c n"))
                    b1_b = ew.tile([P, H], F32)
                    nc.sync.dma_start(b1_b[:], b1[None, :].to_broadcast((P, H)))
                if not trivial_affine_e:
                    eg_b = ew.tile([P, H], F32)
                    nc.sync.dma_start(eg_b[:], e_g[None, :].to_broadcast((P, H)))
                    ebe_b = ew.tile([P, H], F32)
                    nc.sync.dma_start(ebe_b[:], e_be[None, :].to_broadcast((P, H)))

                # ---- U = x@(W0a+W0b)+b0, V = x@W0c   (transposed layout) ----
                u_s = uvp.tile([P, 8, EDGE_ROWS], BF16, tag="u")
                v_s = uvp.tile([P, 8, EDGE_ROWS], BF16, tag="v")
                for m in range(8):
                    for dst, wt, bias in ((u_s, wab_s, True), (v_s, w0c_s, False)):
                        pt = ps.tile([P, H], F32, tag="mm")
                        for half, ncols in ((0, 512), (512, EDGE_ROWS - 512)):
                            for ks in range(4):
                                nc.tensor.matmul(
                                    pt[:, half:half + ncols],
                                    wt[:, ks, m * P:(m + 1) * P],
                                    xT_s[:, ks, half:half + ncols],
                                    start=(ks == 0), stop=(ks == 3),
                                )
                        nc.scalar.activation(
                            dst[:, m, :], pt[:, :EDGE_ROWS], AF.Identity,
                            bias=b0_t[:, m:m + 1] if bias else 0.0,
                        )

                # ---- per-block: build r, edge matmul + LN, aggregate ----
                s_blks = []

                def emit_agg(pagg, ch, z_t):
                    nc.tensor.matmul(pagg[:, 0:512], amat_s[:, ch, :NODES_BLK], z_t[:, 0:512],
                                     start=(ch == 0), stop=(ch == NCHUNK - 1))
                    nc.tensor.matmul(pagg[:, 512:1024], amat_s[:, ch, :NODES_BLK], z_t[:, 512:1024],
                                     start=(ch == 0), stop=(ch == NCHUNK - 1))

                def emit_agg_pair(pagg, cp, zpair):
                    # chunks (2cp, 2cp+1) in one DoubleRow matmul, K=256
                    lhs = amat_s[:, 2 * cp:2 * cp + 2, :NODES_BLK]
                    for half in (0, 512):
                        nc.tensor.matmul(pagg[:, half:half + 512], lhs,
                                         zpair[:, :, half:half + 512],
                                         start=(cp == 0), stop=False, perf_mode=DR)

                def emit_agg_last(pagg, zpair):
                    # chunk 14 alone (sub-slot 0 of the last pair tile), K=128
                    lhs = amat_s[:, NCHUNK - 1, :NODES_BLK]
                    for half in (0, 512):
                        nc.tensor.matmul(pagg[:, half:half + 512], lhs,
                                         zpair[:, 0, half:half + 512],
                                         start=False, stop=True)

                for blk in range(NBLK):
                    r_t = rp.tile([P, 8, E_PAD if FP8 else E_BLK], F8 if FP8 else BF16, tag="r")
                    col0 = blk * NODES_BLK
                    for fs in range(8):
                        u_sl = u_s[:, fs, col0:col0 + NODES_BLK]
                        v_sl = v_s[:, fs, col0:col0 + NODES_BLK]
                        u_in = u_sl.rearrange("p (g i) -> p g i", i=15)[:, :, :, None].to_broadcast((P, GB, 15, 15))
                        v_in = v_sl.rearrange("p (g j) -> p g j", j=15)[:, :, None, :].to_broadcast((P, GB, 15, 15))
                        if FP8:
                            rb = stp.tile([P, E_BLK], BF16, tag="rb")
                            rb_o = rb[:].rearrange("p (g i j) -> p g i j", i=15, j=15)
                            nc.vector.tensor_tensor(rb_o, u_in, v_in, mybir.AluOpType.add)
                            nc.scalar.activation(r_t[:, fs, 0:E_BLK], rb[:], AF.Relu)
                        else:
                            r_o = r_t[:, fs, :].rearrange("p (g i j) -> p g i j", i=15, j=15)
                            nc.vector.tensor_tensor(r_o, u_in, v_in, mybir.AluOpType.add)
                            nc.vector.tensor_scalar_max(r_t[:, fs, :], r_t[:, fs, :], 0.0)

                    pagg = pa.tile([NODES_BLK, H], F32, tag="agg")
                    z_tiles = []
                    for et in range(NCHUNK):
                        m_sz = min(P, E_BLK - et * P)
                        pt = ps.tile([P, H], F32, tag="mm")
                        if FP8:
                            for kp in range(4):
                                lhs = r_t[:, 2 * kp:2 * kp + 2, et * P:et * P + m_sz]
                                nc.tensor.matmul(pt[:m_sz, 0:512], lhs,
                                                 w1_s[:, 2 * kp:2 * kp + 2, 0:512],
                                                 start=(kp == 0), stop=False, perf_mode=DR)
                                nc.tensor.matmul(pt[:m_sz, 512:1024], lhs,
                                                 w1_s[:, 2 * kp:2 * kp + 2, 512:1024],
                                                 start=(kp == 0), stop=False, perf_mode=DR)
                            # bias b1 as a K=1 rank-1 update
                            nc.tensor.matmul(pt[:m_sz, 0:512], ones8[:, :m_sz], b1_r[:, 0:512],
                                             start=False, stop=True)
                            nc.tensor.matmul(pt[:m_sz, 512:1024], ones8[:, :m_sz], b1_r[:, 512:1024],
                                             start=False, stop=True)
                            h1b = pt
                        else:
                            for ks in range(8):
                                lhs = r_t[:, ks, et * P:et * P + m_sz]
                                nc.tensor.matmul(pt[:m_sz, 0:512], lhs, w1_s[:, ks, 0:512],
                                                 start=(ks == 0), stop=(ks == 7))
                                nc.tensor.matmul(pt[:m_sz, 512:1024], lhs, w1_s[:, ks, 512:1024],
                                                 start=(ks == 0), stop=(ks == 7))
                        if FP8:
                            if et % 2 == 0:
                                z_pair = zp.tile([P, 2, H], F8, tag="z")
                                z_tiles.append(z_pair)
                                if m_sz < P:
                                    nc.vector.memset(z_pair[:, 0, :], 0.0)
                            z_t = z_tiles[et // 2][:, et % 2, :]
                        else:
                            z_t = zp.tile([P, H], BF16, tag="z")
                            z_tiles.append(z_t)
                            if m_sz < P:
                                nc.vector.memset(z_t[:], 0.0)
                        # LayerNorm(h1 + b1) then relu; stats read PSUM directly
                        if not FP8:
                            h1b = stp.tile([P, H], F32, tag="h1b")
                            nc.vector.tensor_tensor(h1b[:m_sz], pt[:m_sz], b1_b[:m_sz], mybir.AluOpType.add)
                        st6 = stp.tile([P, 12], F32, tag="st6")
                        nc.vector.bn_stats(st6[:m_sz, 0:6], h1b[:m_sz, 0:512])
                        nc.vector.bn_stats(st6[:m_sz, 6:12], h1b[:m_sz, 512:1024])
                        mv = stp.tile([P, 2], F32, tag="mv")
                        nc.vector.bn_aggr(mv[:m_sz], st6[:m_sz].rearrange("p (a b) -> p a b", b=6))
                        sc = stp.tile([P, 2], F32, tag="sc")
                        nc.scalar.activation(sc[:m_sz, 0:1], mv[:m_sz, 1:2], AF.Sqrt, bias=eps_t[:m_sz])
                        nc.vector.reciprocal(sc[:m_sz, 0:1], sc[:m_sz, 0:1])
                        nc.scalar.activation(sc[:m_sz, 1:2], mv[:m_sz, 0:1], AF.Identity, scale=-1.0)
                        nc.vector.tensor_tensor(sc[:m_sz, 1:2], sc[:m_sz, 1:2], sc[:m_sz, 0:1],
                                                mybir.AluOpType.mult)
                        if trivial_affine_e:
                            nc.scalar.activation(z_t[:m_sz], h1b[:m_sz], AF.Relu,
                                                 bias=sc[:m_sz, 1:2], scale=sc[:m_sz, 0:1])
                        else:
                            zn = stp.tile([P, H], F32, tag="zn")
                            nc.scalar.activation(zn[:m_sz], h1b[:m_sz], AF.Identity,
                                                 bias=sc[:m_sz, 1:2], scale=sc[:m_sz, 0:1])
                            nc.vector.tensor_tensor(zn[:m_sz], zn[:m_sz], eg_b[:m_sz], mybir.AluOpType.mult)
                            nc.vector.tensor_tensor(zn[:m_sz], zn[:m_sz], ebe_b[:m_sz], mybir.AluOpType.add)
                            nc.scalar.activation(z_t[:m_sz], zn[:m_sz], AF.Relu)
                        # interleave aggregation, trailing the LN pipeline
                        if FP8:
                            if et % 2 == 1 and et >= 3:
                                emit_agg_pair(pagg, (et - 3) // 2, z_tiles[(et - 3) // 2])
                        else:
                            if et >= 2:
                                emit_agg(pagg, et - 2, z_tiles[et - 2])
                    if FP8:
                        emit_agg_pair(pagg, 6, z_tiles[6])
                        emit_agg_last(pagg, z_tiles[7])
                    else:
                        emit_agg(pagg, NCHUNK - 2, z_tiles[NCHUNK - 2])
                        emit_agg(pagg, NCHUNK - 1, z_tiles[NCHUNK - 1])

                    # evict aggregated block (transposed into sT at node-phase start)
                    s_blk = cpool.tile([P, H], BF16, tag=f"sblk{blk}")
                    s_blks.append(s_blk)
                    nc.scalar.activation(s_blk[0:NODES_BLK, :], pagg[:], AF.Identity)

            # ================= NODE PHASE =================
            with (
                tc.tile_pool(name="nw", bufs=1) as nw,
                tc.tile_pool(name="nact", bufs=1) as na,
                tc.tile_pool(name="nst", bufs=3) as nst,
                tc.tile_pool(name="ps2", bufs=2, space="PSUM") as ps2,
                tc.tile_pool(name="pa2", bufs=2, space="PSUM") as pa2,
            ):
                nw0x_s = nw.tile([P, 4, H], BF16)
                nc.gpsimd.dma_start(nw0x_s[:], nw0x[:].rearrange("k p n -> p k n"))
                nw0a_s = nw.tile([A_DIM + 1, H], BF16)
                nc.sync.dma_start(nw0a_s[:], nw0a[:])
                nw0s_s = nw.tile([P, 8, H], BF16)
                nc.gpsimd.dma_start(nw0s_s[:], nw0s[:].rearrange("k p n -> p k n"))
                nw1_s = nw.tile([P, 8, H], BF16)
                nc.gpsimd.dma_start(nw1_s[:], nw1[:].rearrange("k p n -> p k n"))
                nw2_s = nw.tile([P, 8, D], BF16)
                nc.gpsimd.dma_start(nw2_s[:], nw2[:].rearrange("k p n -> p k n"))
                nb0_t = nw.tile([P, 8], F32)
                nc.sync.dma_start(nb0_t[:], nb0[:].rearrange("(o p) -> p o", p=P))
                nb1_b = nw.tile([P, H], F32)
                nc.sync.dma_start(nb1_b[:], nb1[None, :].to_broadcast((P, H)))
                nb2_s = nw.tile([1, D], BF16)
                nc.sync.dma_start(nb2_s[:], nb2[:])
                if not trivial_affine_n:
                    ng_b = nw.tile([P, H], F32)
                    nc.sync.dma_start(ng_b[:], n_g[None, :].to_broadcast((P, H)))
                    nbe_b = nw.tile([P, H], F32)
                    nc.sync.dma_start(nbe_b[:], n_be[None, :].to_broadcast((P, H)))

                # ---- transpose aggregated blocks into sT ----
                for blk in range(NBLK):
                    for fs in range(8):
                        ptp = pa2.tile([P, P], BF16, tag="tp")
                        nc.tensor.transpose(
                            ptp[:, 0:NODES_BLK],
                            s_blks[blk][0:NODES_BLK, fs * P:(fs + 1) * P],
                            ident[0:NODES_BLK, 0:NODES_BLK],
                        )
                        nc.scalar.activation(
                            sT[:, fs, blk * NODES_BLK:(blk + 1) * NODES_BLK],
                            ptp[:, 0:NODES_BLK], AF.Identity)

                # ---- node layer 1 -> hT (transposed out, relu+bias in evict) ----
                hT = na.tile([P, 8, N_ROWS], BF16, tag="hT")
                for m in range(8):
                    pt = ps2.tile([P, H], F32, tag="mm")
                    msl = slice(m * P, (m + 1) * P)
                    for half in (0, 512):
                        sl = slice(half, half + 512)
                        chunks = (
                            [(nw0x_s[:, ks, msl], xT_s[:, ks, sl]) for ks in range(4)]
                            + [(nw0a_s[:, msl], actT_s[:, sl])]
                            + [(nw0s_s[:, ks, msl], sT[:, ks, sl]) for ks in range(8)]
                        )
                        for ci, (lhs, rhs) in enumerate(chunks):
                            nc.tensor.matmul(pt[:, sl], lhs, rhs,
                                             start=(ci == 0), stop=(ci == len(chunks) - 1))
                    nc.scalar.activation(hT[:, m, :], pt[:], AF.Relu, bias=nb0_t[:, m:m + 1])

                # ---- node layer 2 (row-major out) + LN + relu -> z2, transpose, layer 3 ----
                z2T = na.tile([P, 8, N_ROWS], BF16, tag="z2T")
                for rt in range(8):
                    pt = ps2.tile([P, H], F32, tag="mm")
                    for ks in range(8):
                        lhs = hT[:, ks, rt * P:(rt + 1) * P]
                        nc.tensor.matmul(pt[:, 0:512], lhs, nw1_s[:, ks, 0:512],
                                         start=(ks == 0), stop=(ks == 7))
                        nc.tensor.matmul(pt[:, 512:1024], lhs, nw1_s[:, ks, 512:1024],
                                         start=(ks == 0), stop=(ks == 7))
                    h2b = nst.tile([P, H], F32, tag="h2b")
                    nc.vector.tensor_tensor(h2b[:], pt[:], nb1_b[:], mybir.AluOpType.add)
                    st6 = nst.tile([P, 12], F32, tag="st6")
                    nc.vector.bn_stats(st6[:, 0:6], h2b[:, 0:512])
                    nc.vector.bn_stats(st6[:, 6:12], h2b[:, 512:1024])
                    mv = nst.tile([P, 2], F32, tag="mv")
                    nc.vector.bn_aggr(mv[:], st6[:].rearrange("p (a b) -> p a b", b=6))
                    sc = nst.tile([P, 2], F32, tag="sc")
                    nc.scalar.activation(sc[:, 0:1], mv[:, 1:2], AF.Sqrt, bias=eps_t[:])
                    nc.vector.reciprocal(sc[:, 0:1], sc[:, 0:1])
                    nc.scalar.activation(sc[:, 1:2], mv[:, 0:1], AF.Identity, scale=-1.0)
                    nc.vector.tensor_tensor(sc[:, 1:2], sc[:, 1:2], sc[:, 0:1], mybir.AluOpType.mult)
                    z2 = nst.tile([P, H], BF16, tag="z2")
                    if trivial_affine_n:
                        nc.scalar.activation(z2[:], h2b[:], AF.Relu,
                                             bias=sc[:, 1:2], scale=sc[:, 0:1])
                    else:
                        zn = nst.tile([P, H], F32, tag="zn")
                        nc.scalar.activation(zn[:], h2b[:], AF.Identity,
                                             bias=sc[:, 1:2], scale=sc[:, 0:1])
                        nc.vector.tensor_tensor(zn[:], zn[:], ng_b[:], mybir.AluOpType.mult)
                        nc.vector.tensor_tensor(zn[:], zn[:], nbe_b[:], mybir.AluOpType.add)
                        nc.scalar.activation(z2[:], zn[:], AF.Relu)
                    for fs in range(8):
                        ptp = pa2.tile([P, P], BF16, tag="tp")
                        nc.tensor.transpose(ptp[:], z2[:, fs * P:(fs + 1) * P], ident[:])
                        nc.scalar.activation(z2T[:, fs, rt * P:(rt + 1) * P], ptp[:], AF.Identity)

                # ---- node layer 3 + bias ----
                out_r = out[:].rearrange("(rt p) d -> p rt d", p=P)
                for rt in range(8):
                    pt = ps2.tile([P, H], F32, tag="mm")
                    for ks in range(8):
                        nc.tensor.matmul(pt[:, 0:D], z2T[:, ks, rt * P:(rt + 1) * P],
                                         nw2_s[:, ks, :], start=(ks == 0), stop=False)
                    nc.tensor.matmul(pt[:, 0:D], ones_row[:], nb2_s[:], start=False, stop=True)
                    outb = nst.tile([P, D], F32, tag="outb")
                    nc.scalar.activation(outb[:], pt[:, 0:D], AF.Identity)
                    nc.sync.dma_start(out_r[:, rt, :], outb[:])

    return nc


_PROG_CACHE = {}


def _get_program(trivial_e, trivial_n):
    key = (trivial_e, trivial_n, FP8)
    if key not in _PROG_CACHE:
        nc = _build_program(trivial_e, trivial_n)
        nc.finalize()
        _PROG_CACHE[key] = nc
    return _PROG_CACHE[key]


def kernel(states, action, e_w0, e_b0, e_w1, e_b1, e_g, e_be, e_w2, e_b2,
           n_w0, n_b0, n_w1, n_b1, n_g, n_be, n_w2, n_b2):
    states = _f32(states)
    action = np.asarray(action).astype(np.int64)
    e_w0, e_b0, e_w1, e_b1 = _f32(e_w0), _f32(e_b0), _f32(e_w1), _f32(e_b1)
    e_g, e_be, e_w2, e_b2 = _f32(e_g), _f32(e_be), _f32(e_w2), _f32(e_b2)
    n_w0, n_b0, n_w1, n_b1 = _f32(n_w0), _f32(n_b0), _f32(n_w1), _f32(n_b1)
    n_g, n_be, n_w2, n_b2 = _f32(n_g), _f32(n_be), _f32(n_w2), _f32(n_b2)

    trivial_e = bool(np.all(e_g == 1.0) and np.all(e_be == 0.0))
    trivial_n = bool(np.all(n_g == 1.0) and np.all(n_be == 0.0))
    nc = _get_program(trivial_e, trivial_n)

    flat = states.reshape(-1, D)                        # [8192, 512]
    # one-hot action vectors per flat row
    av = np.zeros((B, A_DIM * K), dtype=np.float32)
    av[np.arange(B), action] = 1.0
    av = av.reshape(-1, A_DIM)                          # [8192, 20]

    # host-folded weights
    wab = e_w0[0:D] + e_w0[D:2 * D]                     # [512, 1024]
    w0c = e_w0[2 * D:3 * D]
    nw0x = n_w0[0:D]
    nw0a = n_w0[D:D + A_DIM]
    n_w0s_part = n_w0[D + A_DIM:]
    nw0s = e_w2 @ n_w0s_part                            # [1024, 1024]
    nb0 = n_b0
    nw0a21 = np.concatenate([nw0a, (e_b2 @ n_w0s_part).reshape(1, H)], axis=0)

    amat = _build_amat()

    def kslice(w, kt):   # [K, N] -> [K/128, 128, N]
        return w.reshape(kt, P, w.shape[1])

    if FP8:
        amat_in = _f8(np.concatenate(
            [amat.reshape(NCHUNK, P, NODES_BLK),
             np.zeros((NCHUNK, P, P - NODES_BLK), np.float32)], axis=2))
        w1_in = _f8(kslice(e_w1, 8))
        b1_in = _f8(e_b1.reshape(1, H))
    else:
        amat_in = _bf16(amat.reshape(NCHUNK, P, NODES_BLK))
        w1_in = _bf16(kslice(e_w1, 8))
        b1_in = _f32(e_b1)
    common = {
        "wab": _bf16(kslice(wab, 4)), "w0c": _bf16(kslice(w0c, 4)),
        "b0": _f32(e_b0), "w1": w1_in, "b1": b1_in,
        "amat": amat_in,
        "nw0x": _bf16(kslice(nw0x, 4)), "nw0a": _bf16(nw0a21),
        "nw0s": _bf16(kslice(nw0s, 8)), "nb0": _f32(nb0),
        "nw1": _bf16(kslice(n_w1, 8)), "nb1": _f32(n_b1),
        "nw2": _bf16(kslice(n_w2, 8)), "nb2": _bf16(n_b2.reshape(1, D)),
    }
    if not trivial_e:
        common["e_g"] = _f32(e_g)
        common["e_be"] = _f32(e_be)
    if not trivial_n:
        common["n_g"] = _f32(n_g)
        common["n_be"] = _f32(n_be)

    in_maps = []
    row_idx = []
    for c in range(N_CORES):
        idx = np.concatenate([
            np.arange(c * EDGE_ROWS, (c + 1) * EDGE_ROWS),
            np.arange(NG * 15 + c * EXTRA_ROWS, NG * 15 + (c + 1) * EXTRA_ROWS),
        ])
        row_idx.append(idx)
        x_rows = flat[idx]                              # [1024, 512]
        xt = np.ascontiguousarray(x_rows.T)             # [512, 1024]
        at = np.concatenate([av[idx].T, np.concatenate(
            [np.full((1, EDGE_ROWS), 14.0, np.float32),
             np.zeros((1, EXTRA_ROWS), np.float32)], axis=1)], axis=0)  # [21, 1024]
        m = dict(common)
        m["xT"] = _bf16(xt.reshape(4, P, N_ROWS))
        m["actT"] = _bf16(at)
        in_maps.append(m)

    res = run_bass_kernel_spmd(nc, in_maps, core_ids=list(range(N_CORES)))
    global LAST_RESULT
    LAST_RESULT = res

    out_full = np.empty((B * K, D), dtype=np.float32)
    for c in range(N_CORES):
        out_full[row_idx[c]] = flat[row_idx[c]] + res.results[c]["out"]
    return out_full.reshape(B, K, D)


# revision 20
# speedup vs baseline: 1.0371x; 1.0371x over previous
"""CSWM transition GNN kernel for 8 TRN2 NeuronCores.

Sharding: data-parallel over the 512 edge-groups (the quirky edge list is
block-diagonal over groups of 15 consecutive flat rows). Each core gets
64 groups (960 edge rows) + 64 of the 512 zero-agg tail rows = 1024 node
rows. No cross-core communication.

Host-side algebra:
  - cat(xi,xi,xj)@e_w0 = xi@(W0a+W0b) + xj@W0c          (per-node U,V)
  - final edge matmul commutes with scatter-add; W2 then folds into the
    node MLP first layer: nw0s = e_w2 @ n_w0[532:1556]
  - per-edge work: one 1024x1024 matmul + LayerNorm + relu
"""

import numpy as np
import ml_dtypes

import concourse.bass as bass
import concourse.mybir as mybir
import concourse.tile as tile
from concourse import bacc
from concourse.bass_utils import run_bass_kernel_spmd
from concourse.masks import make_identity

BF16 = mybir.dt.bfloat16
F32 = mybir.dt.float32
F8 = mybir.dt.float8e4
DR = mybir.MatmulPerfMode.DoubleRow
AF = mybir.ActivationFunctionType

P = 128
D = 512            # embedding dim
H = 1024           # hidden dim
A_DIM = 20         # action dim
B = 512            # batch
K = 16             # objects
NG = 512           # total edge groups (block-diag over 15-row groups)
N_CORES = 8
G_CORE = NG // N_CORES          # 64 groups per core
EDGE_ROWS = G_CORE * 15         # 960
EXTRA_ROWS = (B * K - NG * 15) // N_CORES   # 64 zero-agg tail rows per core
N_ROWS = EDGE_ROWS + EXTRA_ROWS  # 1024 node rows per core
GB = 8                          # groups per aggregation block
NBLK = G_CORE // GB             # 8 blocks per core
E_BLK = GB * 225                # 1800 edges per block (incl. diagonal)
NCHUNK = (E_BLK + P - 1) // P   # 15 chunks of 128 edge-slots
NODES_BLK = GB * 15             # 120
E_PAD = 1808                    # E_BLK padded so fp8 DoubleRow k-pair stride %16==0
EPS = 1e-5
FP8 = True                      # fp8e4m3 DoubleRow for the edge matmul + aggregation


def _bf16(x):
    return np.ascontiguousarray(np.asarray(x, dtype=np.float32).astype(ml_dtypes.bfloat16))


def _f8(x):
    return np.ascontiguousarray(np.asarray(x, dtype=np.float32).astype(ml_dtypes.float8_e4m3))


def _f32(x):
    return np.ascontiguousarray(np.asarray(x, dtype=np.float32))


def _build_amat():
    """[NCHUNK*128, 120] 0/1 matrix: edge (gb,i,j) -> node gb*15+i, diagonal
    (j==i) excluded, padding rows zero."""
    a = np.zeros((NCHUNK * P, NODES_BLK), dtype=np.float32)
    for gb in range(GB):
        for i in range(15):
            for j in range(15):
                if i != j:
                    a[gb * 225 + i * 15 + j, gb * 15 + i] = 1.0
    return a


def _build_program(trivial_affine_e: bool, trivial_affine_n: bool):
    nc = bacc.Bacc("TRN2", target_bir_lowering=False, debug=False)

    # ---- DRAM parameters (per-core shards / replicated weights) ----
    def din(name, shape, dt):
        return nc.declare_dram_parameter(name, list(shape), dt, isOutput=False)

    xT = din("xT", (4, P, N_ROWS), BF16)       # x transposed, [ks,p,rows]
    actT = din("actT", (A_DIM + 1, N_ROWS), BF16)   # one-hot actions + edge-row indicator
    wab = din("wab", (4, P, H), BF16)          # W0a+W0b  [ks,p,out]
    w0c = din("w0c", (4, P, H), BF16)
    b0 = din("b0", (H,), F32)
    if FP8:
        w1 = din("w1", (8, P, H), F8)
        b1 = din("b1", (1, H), F8)
        amat = din("amat", (NCHUNK, P, P), F8)
    else:
        w1 = din("w1", (8, P, H), BF16)
        b1 = din("b1", (H,), F32)
        amat = din("amat", (NCHUNK, P, NODES_BLK), BF16)
    nw0x = din("nw0x", (4, P, H), BF16)
    nw0a = din("nw0a", (A_DIM + 1, H), BF16)   # rows 0..19 action, row 20 = e_b2 @ n_w0s
    nw0s = din("nw0s", (8, P, H), BF16)
    nb0 = din("nb0", (H,), F32)
    nw1 = din("nw1", (8, P, H), BF16)
    nb1 = din("nb1", (H,), F32)
    nw2 = din("nw2", (8, P, D), BF16)
    nb2 = din("nb2", (1, D), BF16)
    if not trivial_affine_e:
        e_g = din("e_g", (H,), F32)
        e_be = din("e_be", (H,), F32)
    if not trivial_affine_n:
        n_g = din("n_g", (H,), F32)
        n_be = din("n_be", (H,), F32)

    out = nc.declare_dram_parameter("out", [N_ROWS, D], F32, isOutput=True)

    with tile.TileContext(nc) as tc:
        with tc.tile_pool(name="const", bufs=1) as cpool:
            xT_s = cpool.tile([P, 4, N_ROWS], BF16)
            for ks in range(4):
                nc.sync.dma_start(xT_s[:, ks, :], xT[ks])
            actT_s = cpool.tile([A_DIM + 1, N_ROWS], BF16)
            nc.sync.dma_start(actT_s[:], actT[:])
            ident = cpool.tile([P, P], BF16)
            make_identity(nc, ident)
            ones_row = cpool.tile([1, P], BF16)
            nc.vector.memset(ones_row[:], 1.0)
            eps_t = cpool.tile([P, 1], F32)
            nc.vector.memset(eps_t[:], EPS)
            # sT: aggregated-hidden, transposed [feat, rows]; tail rows zero
            sT = cpool.tile([P, 8, N_ROWS], BF16)
            nc.vector.memset(sT[:, :, EDGE_ROWS:N_ROWS], 0.0)

            # ================= EDGE PHASE =================
            with (
                tc.tile_pool(name="ew", bufs=1) as ew,
                tc.tile_pool(name="uv", bufs=1) as uvp,
                tc.tile_pool(name="rp", bufs=2) as rp,
                tc.tile_pool(name="zp", bufs=6) as zp,
                tc.tile_pool(name="st", bufs=2) as stp,
                tc.tile_pool(name="ps", bufs=3 if FP8 else 2, space="PSUM") as ps,
                tc.tile_pool(name="pa", bufs=1 if FP8 else 2, space="PSUM") as pa,
            ):
                wab_s = ew.tile([P, 4, H], BF16)
                nc.sync.dma_start(wab_s[:], wab[:].rearrange("k p n -> p k n"))
                w0c_s = ew.tile([P, 4, H], BF16)
                nc.sync.dma_start(w0c_s[:], w0c[:].rearrange("k p n -> p k n"))
                b0_t = ew.tile([P, 8], F32)
                nc.sync.dma_start(b0_t[:], b0[:].rearrange("(o p) -> p o", p=P))
                if FP8:
                    w1_s = ew.tile([P, 8, H], F8)
                    nc.gpsimd.dma_start(w1_s[:], w1[:].rearrange("k p n -> p k n"))
                    amat_s = ew.tile([P, NCHUNK, P], F8)
                    nc.gpsimd.dma_start(amat_s[:], amat[:].rearrange("c p n -> p c n"))
                    b1_r = ew.tile([1, H], F8)
                    nc.sync.dma_start(b1_r[:], b1[:])
                    ones8 = ew.tile([1, P], F8)
                    nc.vector.memset(ones8[:], 1.0)
                else:
                    w1_s = ew.tile([P, 8, H], BF16)
                    nc.sync.dma_start(w1_s[:], w1[:].rearrange("k p n -> p k n"))
                    amat_s = ew.tile([P, NCHUNK, NODES_BLK], BF16)
                    nc.sync.dma_start(amat_s[:], amat[:].rearrange("c p n -> p c n"))
                    b1_b = ew.tile([P, H], F32)
                    nc.sync.dma_start(b1_b[:], b1[None, :].to_broadcast((P, H)))
                if not trivial_affine_e:
                    eg_b = ew.tile([P, H], F32)
                    nc.sync.dma_start(eg_b[:], e_g[None, :].to_broadcast((P, H)))
                    ebe_b = ew.tile([P, H], F32)
                    nc.sync.dma_start(ebe_b[:], e_be[None, :].to_broadcast((P, H)))

                # ---- U = x@(W0a+W0b)+b0, V = x@W0c   (transposed layout) ----
                u_s = uvp.tile([P, 8, EDGE_ROWS], BF16, tag="u")
                v_s = uvp.tile([P, 8, EDGE_ROWS], BF16, tag="v")
                for m in range(8):
                    for dst, wt, bias in ((u_s, wab_s, True), (v_s, w0c_s, False)):
                        pt = ps.tile([P, H], F32, tag="mm")
                        for half, ncols in ((0, 512), (512, EDGE_ROWS - 512)):
                            for ks in range(4):
                                nc.tensor.matmul(
                                    pt[:, half:half + ncols],
                                    wt[:, ks, m * P:(m + 1) * P],
                                    xT_s[:, ks, half:half + ncols],
                                    start=(ks == 0), stop=(ks == 3),
                                )
                        nc.scalar.activation(
                            dst[:, m, :], pt[:, :EDGE_ROWS], AF.Identity,
                            bias=b0_t[:, m:m + 1] if bias else 0.0,
                        )

                # ---- per-block: build r, edge matmul + LN, aggregate ----
                s_blks = []

                def emit_agg(pagg, ch, z_t):
                    nc.tensor.matmul(pagg[:, 0:512], amat_s[:, ch, :NODES_BLK], z_t[:, 0:512],
                                     start=(ch == 0), stop=(ch == NCHUNK - 1))
                    nc.tensor.matmul(pagg[:, 512:1024], amat_s[:, ch, :NODES_BLK], z_t[:, 512:1024],
                                     start=(ch == 0), stop=(ch == NCHUNK - 1))

                def emit_agg_pair(pagg, cp, zpair):
                    # chunks (2cp, 2cp+1) in one DoubleRow matmul, K=256
                    lhs = amat_s[:, 2 * cp:2 * cp + 2, :NODES_BLK]
                    for half in (0, 512):
                        nc.tensor.matmul(pagg[:, half:half + 512], lhs,
                                         zpair[:, :, half:half + 512],
                                         start=(cp == 0), stop=False, perf_mode=DR)

                def emit_agg_last(pagg, zpair):
                    # chunk 14 alone (sub-slot 0 of the last pair tile), K=128
                    lhs = amat_s[:, NCHUNK - 1, :NODES_BLK]
                    for half in (0, 512):
                        nc.tensor.matmul(pagg[:, half:half + 512], lhs,
                                         zpair[:, 0, half:half + 512],
                                         start=False, stop=True)

                for blk in range(NBLK):
                    r_t = rp.tile([P, 8, E_PAD if FP8 else E_BLK], F8 if FP8 else BF16, tag="r")
                    col0 = blk * NODES_BLK
                    for fs in range(8):
                        u_sl = u_s[:, fs, col0:col0 + NODES_BLK]
                        v_sl = v_s[:, fs, col0:col0 + NODES_BLK]
                        u_in = u_sl.rearrange("p (g i) -> p g i", i=15)[:, :, :, None].to_broadcast((P, GB, 15, 15))
                        v_in = v_sl.rearrange("p (g j) -> p g j", j=15)[:, :, None, :].to_broadcast((P, GB, 15, 15))
                        if FP8:
                            rb = stp.tile([P, E_BLK], BF16, tag="rb")
                            rb_o = rb[:].rearrange("p (g i j) -> p g i j", i=15, j=15)
                            nc.vector.tensor_tensor(rb_o, u_in, v_in, mybir.AluOpType.add)
                            nc.scalar.activation(r_t[:, fs, 0:E_BLK], rb[:], AF.Relu)
                        else:
                            r_o = r_t[:, fs, :].rearrange("p (g i j) -> p g i j", i=15, j=15)
                            nc.vector.tensor_tensor(r_o, u_in, v_in, mybir.AluOpType.add)
                            nc.vector.tensor_scalar_max(r_t[:, fs, :], r_t[:, fs, :], 0.0)

                    pagg = pa.tile([NODES_BLK, H], F32, tag="agg")
                    z_tiles = []
                    for et in range(NCHUNK):
                        m_sz = min(P, E_BLK - et * P)
                        pt = ps.tile([P, H], F32, tag="mm")
                        if FP8:
                            for kp in range(4):
                                lhs = r_t[:, 2 * kp:2 * kp + 2, et * P:et * P + m_sz]
                                nc.tensor.matmul(pt[:m_sz, 0:512], lhs,
                                                 w1_s[:, 2 * kp:2 * kp + 2, 0:512],
                                                 start=(kp == 0), stop=False, perf_mode=DR)
                                nc.tensor.matmul(pt[:m_sz, 512:1024], lhs,
                                                 w1_s[:, 2 * kp:2 * kp + 2, 512:1024],
                                                 start=(kp == 0), stop=False, perf_mode=DR)
                            # bias b1 as a K=1 rank-1 update
                            nc.tensor.matmul(pt[:m_sz, 0:512], ones8[:, :m_sz], b1_r[:, 0:512],
                                             start=False, stop=True)
                            nc.tensor.matmul(pt[:m_sz, 512:1024], ones8[:, :m_sz], b1_r[:, 512:1024],
                                             start=False, stop=True)
                            h1b = pt
                        else:
                            for ks in range(8):
                                lhs = r_t[:, ks, et * P:et * P + m_sz]
                                nc.tensor.matmul(pt[:m_sz, 0:512], lhs, w1_s[:, ks, 0:512],
                                                 start=(ks == 0), stop=(ks == 7))
                                nc.tensor.matmul(pt[:m_sz, 512:1024], lhs, w1_s[:, ks, 512:1024],
                                                 start=(ks == 0), stop=(ks == 7))
                        if FP8:
                            if et % 2 == 0:
                                z_pair = zp.tile([P, 2, H], F8, tag="z")
                                z_tiles.append(z_pair)
                                if m_sz < P:
                                    nc.vector.memset(z_pair[:, 0, :], 0.0)
                            z_t = z_tiles[et // 2][:, et % 2, :]
                        else:
                            z_t = zp.tile([P, H], BF16, tag="z")
                            z_tiles.append(z_t)
                            if m_sz < P:
                                nc.vector.memset(z_t[:], 0.0)
                        # LayerNorm(h1 + b1) then relu; stats read PSUM directly
                        if not FP8:
                            h1b = stp.tile([P, H], F32, tag="h1b")
                            nc.vector.tensor_tensor(h1b[:m_sz], pt[:m_sz], b1_b[:m_sz], mybir.AluOpType.add)
                        st6 = stp.tile([P, 12], F32, tag="st6")
                        nc.vector.bn_stats(st6[:m_sz, 0:6], h1b[:m_sz, 0:512])
                        nc.vector.bn_stats(st6[:m_sz, 6:12], h1b[:m_sz, 512:1024])
                        mv = stp.tile([P, 2], F32, tag="mv")
                        nc.vector.bn_aggr(mv[:m_sz], st6[:m_sz].rearrange("p (a b) -> p a b", b=6))
                        sc = stp.tile([P, 2], F32, tag="sc")
                        nc.scalar.activation(sc[:m_sz, 0:1], mv[:m_sz, 1:2],
                                             AF.Abs_reciprocal_sqrt, bias=eps_t[:m_sz])
                        nc.vector.tensor_scalar(sc[:m_sz, 1:2], mv[:m_sz, 0:1],
                                                sc[:m_sz, 0:1], -1.0,
                                                mybir.AluOpType.mult, mybir.AluOpType.mult)
                        if trivial_affine_e:
                            nc.scalar.activation(z_t[:m_sz], h1b[:m_sz], AF.Relu,
                                                 bias=sc[:m_sz, 1:2], scale=sc[:m_sz, 0:1])
                        else:
                            zn = stp.tile([P, H], F32, tag="zn")
                            nc.scalar.activation(zn[:m_sz], h1b[:m_sz], AF.Identity,
                                                 bias=sc[:m_sz, 1:2], scale=sc[:m_sz, 0:1])
                            nc.vector.tensor_tensor(zn[:m_sz], zn[:m_sz], eg_b[:m_sz], mybir.AluOpType.mult)
                            nc.vector.tensor_tensor(zn[:m_sz], zn[:m_sz], ebe_b[:m_sz], mybir.AluOpType.add)
                            nc.scalar.activation(z_t[:m_sz], zn[:m_sz], AF.Relu)
                        # interleave aggregation, trailing the LN pipeline
                        if FP8:
                            if et % 2 == 1 and et >= 3:
                                emit_agg_pair(pagg, (et - 3) // 2, z_tiles[(et - 3) // 2])
                        else:
                            if et >= 2:
                                emit_agg(pagg, et - 2, z_tiles[et - 2])
                    if FP8:
                        emit_agg_pair(pagg, 6, z_tiles[6])
                        emit_agg_last(pagg, z_tiles[7])
                    else:
                        emit_agg(pagg, NCHUNK - 2, z_tiles[NCHUNK - 2])
                        emit_agg(pagg, NCHUNK - 1, z_tiles[NCHUNK - 1])

                    # evict aggregated block (transposed into sT at node-phase start)
                    s_blk = cpool.tile([P, H], BF16, tag=f"sblk{blk}")
                    s_blks.append(s_blk)
                    nc.scalar.activation(s_blk[0:NODES_BLK, :], pagg[:], AF.Identity)

            # ================= NODE PHASE =================
            with (
                tc.tile_pool(name="nw", bufs=1) as nw,
                tc.tile_pool(name="nact", bufs=1) as na,
                tc.tile_pool(name="nst", bufs=3) as nst,
                tc.tile_pool(name="ps2", bufs=2, space="PSUM") as ps2,
                tc.tile_pool(name="pa2", bufs=2, space="PSUM") as pa2,
            ):
                nw0x_s = nw.tile([P, 4, H], BF16)
                nc.gpsimd.dma_start(nw0x_s[:], nw0x[:].rearrange("k p n -> p k n"))
                nw0a_s = nw.tile([A_DIM + 1, H], BF16)
                nc.sync.dma_start(nw0a_s[:], nw0a[:])
                nw0s_s = nw.tile([P, 8, H], BF16)
                nc.gpsimd.dma_start(nw0s_s[:], nw0s[:].rearrange("k p n -> p k n"))
                nw1_s = nw.tile([P, 8, H], BF16)
                nc.gpsimd.dma_start(nw1_s[:], nw1[:].rearrange("k p n -> p k n"))
                nw2_s = nw.tile([P, 8, D], BF16)
                nc.gpsimd.dma_start(nw2_s[:], nw2[:].rearrange("k p n -> p k n"))
                nb0_t = nw.tile([P, 8], F32)
                nc.sync.dma_start(nb0_t[:], nb0[:].rearrange("(o p) -> p o", p=P))
                nb1_b = nw.tile([P, H], F32)
                nc.sync.dma_start(nb1_b[:], nb1[None, :].to_broadcast((P, H)))
                nb2_s = nw.tile([1, D], BF16)
                nc.sync.dma_start(nb2_s[:], nb2[:])
                if not trivial_affine_n:
                    ng_b = nw.tile([P, H], F32)
                    nc.sync.dma_start(ng_b[:], n_g[None, :].to_broadcast((P, H)))
                    nbe_b = nw.tile([P, H], F32)
                    nc.sync.dma_start(nbe_b[:], n_be[None, :].to_broadcast((P, H)))

                # ---- transpose aggregated blocks into sT ----
                for blk in range(NBLK):
                    for fs in range(8):
                        ptp = pa2.tile([P, P], BF16, tag="tp")
                        nc.tensor.transpose(
                            ptp[:, 0:NODES_BLK],
                            s_blks[blk][0:NODES_BLK, fs * P:(fs + 1) * P],
                            ident[0:NODES_BLK, 0:NODES_BLK],
                        )
                        nc.scalar.activation(
                            sT[:, fs, blk * NODES_BLK:(blk + 1) * NODES_BLK],
                            ptp[:, 0:NODES_BLK], AF.Identity)

                # ---- node layer 1 -> hT (transposed out, relu+bias in evict) ----
                hT = na.tile([P, 8, N_ROWS], BF16, tag="hT")
                for m in range(8):
                    pt = ps2.tile([P, H], F32, tag="mm")
                    msl = slice(m * P, (m + 1) * P)
                    for half in (0, 512):
                        sl = slice(half, half + 512)
                        chunks = (
                            [(nw0x_s[:, ks, msl], xT_s[:, ks, sl]) for ks in range(4)]
                            + [(nw0a_s[:, msl], actT_s[:, sl])]
                            + [(nw0s_s[:, ks, msl], sT[:, ks, sl]) for ks in range(8)]
                        )
                        for ci, (lhs, rhs) in enumerate(chunks):
                            nc.tensor.matmul(pt[:, sl], lhs, rhs,
                                             start=(ci == 0), stop=(ci == len(chunks) - 1))
                    nc.scalar.activation(hT[:, m, :], pt[:], AF.Relu, bias=nb0_t[:, m:m + 1])

                # ---- node layer 2 (row-major out) + LN + relu -> z2, transpose, layer 3 ----
                z2T = na.tile([P, 8, N_ROWS], BF16, tag="z2T")
                for rt in range(8):
                    pt = ps2.tile([P, H], F32, tag="mm")
                    for ks in range(8):
                        lhs = hT[:, ks, rt * P:(rt + 1) * P]
                        nc.tensor.matmul(pt[:, 0:512], lhs, nw1_s[:, ks, 0:512],
                                         start=(ks == 0), stop=(ks == 7))
                        nc.tensor.matmul(pt[:, 512:1024], lhs, nw1_s[:, ks, 512:1024],
                                         start=(ks == 0), stop=(ks == 7))
                    h2b = nst.tile([P, H], F32, tag="h2b")
                    nc.vector.tensor_tensor(h2b[:], pt[:], nb1_b[:], mybir.AluOpType.add)
                    st6 = nst.tile([P, 12], F32, tag="st6")
                    nc.vector.bn_stats(st6[:, 0:6], h2b[:, 0:512])
                    nc.vector.bn_stats(st6[:, 6:12], h2b[:, 512:1024])
                    mv = nst.tile([P, 2], F32, tag="mv")
                    nc.vector.bn_aggr(mv[:], st6[:].rearrange("p (a b) -> p a b", b=6))
                    sc = nst.tile([P, 2], F32, tag="sc")
                    nc.scalar.activation(sc[:, 0:1], mv[:, 1:2],
                                         AF.Abs_reciprocal_sqrt, bias=eps_t[:])
                    nc.vector.tensor_scalar(sc[:, 1:2], mv[:, 0:1], sc[:, 0:1], -1.0,
                                            mybir.AluOpType.mult, mybir.AluOpType.mult)
                    z2 = nst.tile([P, H], BF16, tag="z2")
                    if trivial_affine_n:
                        nc.scalar.activation(z2[:], h2b[:], AF.Relu,
                                             bias=sc[:, 1:2], scale=sc[:, 0:1])
                    else:
                        zn = nst.tile([P, H], F32, tag="zn")
                        nc.scalar.activation(zn[:], h2b[:], AF.Identity,
                                             bias=sc[:, 1:2], scale=sc[:, 0:1])
                        nc.vector.tensor_tensor(zn[:], zn[:], ng_b[:], mybir.AluOpType.mult)
                        nc.vector.tensor_tensor(zn[:], zn[:], nbe_b[:], mybir.AluOpType.add)
                        nc.scalar.activation(z2[:], zn[:], AF.Relu)
                    for fs in range(8):
                        ptp = pa2.tile([P, P], BF16, tag="tp")
                        nc.tensor.transpose(ptp[:], z2[:, fs * P:(fs + 1) * P], ident[:])
                        nc.scalar.activation(z2T[:, fs, rt * P:(rt + 1) * P], ptp[:], AF.Identity)

                # ---- node layer 3 + bias ----
                out_r = out[:].rearrange("(rt p) d -> p rt d", p=P)
                for rt in range(8):
                    pt = ps2.tile([P, H], F32, tag="mm")
                    for ks in range(8):
                        nc.tensor.matmul(pt[:, 0:D], z2T[:, ks, rt * P:(rt + 1) * P],
                                         nw2_s[:, ks, :], start=(ks == 0), stop=False)
                    nc.tensor.matmul(pt[:, 0:D], ones_row[:], nb2_s[:], start=False, stop=True)
                    outb = nst.tile([P, D], F32, tag="outb")
                    nc.scalar.activation(outb[:], pt[:, 0:D], AF.Identity)
                    nc.sync.dma_start(out_r[:, rt, :], outb[:])

    return nc


_PROG_CACHE = {}


def _get_program(trivial_e, trivial_n):
    key = (trivial_e, trivial_n, FP8)
    if key not in _PROG_CACHE:
        nc = _build_program(trivial_e, trivial_n)
        nc.finalize()
        _PROG_CACHE[key] = nc
    return _PROG_CACHE[key]


def kernel(states, action, e_w0, e_b0, e_w1, e_b1, e_g, e_be, e_w2, e_b2,
           n_w0, n_b0, n_w1, n_b1, n_g, n_be, n_w2, n_b2):
    states = _f32(states)
    action = np.asarray(action).astype(np.int64)
    e_w0, e_b0, e_w1, e_b1 = _f32(e_w0), _f32(e_b0), _f32(e_w1), _f32(e_b1)
    e_g, e_be, e_w2, e_b2 = _f32(e_g), _f32(e_be), _f32(e_w2), _f32(e_b2)
    n_w0, n_b0, n_w1, n_b1 = _f32(n_w0), _f32(n_b0), _f32(n_w1), _f32(n_b1)
    n_g, n_be, n_w2, n_b2 = _f32(n_g), _f32(n_be), _f32(n_w2), _f32(n_b2)

    trivial_e = bool(np.all(e_g == 1.0) and np.all(e_be == 0.0))
    trivial_n = bool(np.all(n_g == 1.0) and np.all(n_be == 0.0))
    nc = _get_program(trivial_e, trivial_n)

    flat = states.reshape(-1, D)                        # [8192, 512]
    # one-hot action vectors per flat row
    av = np.zeros((B, A_DIM * K), dtype=np.float32)
    av[np.arange(B), action] = 1.0
    av = av.reshape(-1, A_DIM)                          # [8192, 20]

    # host-folded weights
    wab = e_w0[0:D] + e_w0[D:2 * D]                     # [512, 1024]
    w0c = e_w0[2 * D:3 * D]
    nw0x = n_w0[0:D]
    nw0a = n_w0[D:D + A_DIM]
    n_w0s_part = n_w0[D + A_DIM:]
    nw0s = e_w2 @ n_w0s_part                            # [1024, 1024]
    nb0 = n_b0
    nw0a21 = np.concatenate([nw0a, (e_b2 @ n_w0s_part).reshape(1, H)], axis=0)

    amat = _build_amat()

    def kslice(w, kt):   # [K, N] -> [K/128, 128, N]
        return w.reshape(kt, P, w.shape[1])

    if FP8:
        amat_in = _f8(np.concatenate(
            [amat.reshape(NCHUNK, P, NODES_BLK),
             np.zeros((NCHUNK, P, P - NODES_BLK), np.float32)], axis=2))
        w1_in = _f8(kslice(e_w1, 8))
        b1_in = _f8(e_b1.reshape(1, H))
    else:
        amat_in = _bf16(amat.reshape(NCHUNK, P, NODES_BLK))
        w1_in = _bf16(kslice(e_w1, 8))
        b1_in = _f32(e_b1)
    common = {
        "wab": _bf16(kslice(wab, 4)), "w0c": _bf16(kslice(w0c, 4)),
        "b0": _f32(e_b0), "w1": w1_in, "b1": b1_in,
        "amat": amat_in,
        "nw0x": _bf16(kslice(nw0x, 4)), "nw0a": _bf16(nw0a21),
        "nw0s": _bf16(kslice(nw0s, 8)), "nb0": _f32(nb0),
        "nw1": _bf16(kslice(n_w1, 8)), "nb1": _f32(n_b1),
        "nw2": _bf16(kslice(n_w2, 8)), "nb2": _bf16(n_b2.reshape(1, D)),
    }
    if not trivial_e:
        common["e_g"] = _f32(e_g)
        common["e_be"] = _f32(e_be)
    if not trivial_n:
        common["n_g"] = _f32(n_g)
        common["n_be"] = _f32(n_be)

    in_maps = []
    row_idx = []
    for c in range(N_CORES):
        idx = np.concatenate([
            np.arange(c * EDGE_ROWS, (c + 1) * EDGE_ROWS),
            np.arange(NG * 15 + c * EXTRA_ROWS, NG * 15 + (c + 1) * EXTRA_ROWS),
        ])
        row_idx.append(idx)
        x_rows = flat[idx]                              # [1024, 512]
        xt = np.ascontiguousarray(x_rows.T)             # [512, 1024]
        at = np.concatenate([av[idx].T, np.concatenate(
            [np.full((1, EDGE_ROWS), 14.0, np.float32),
             np.zeros((1, EXTRA_ROWS), np.float32)], axis=1)], axis=0)  # [21, 1024]
        m = dict(common)
        m["xT"] = _bf16(xt.reshape(4, P, N_ROWS))
        m["actT"] = _bf16(at)
        in_maps.append(m)

    res = run_bass_kernel_spmd(nc, in_maps, core_ids=list(range(N_CORES)))
    global LAST_RESULT
    LAST_RESULT = res

    out_full = np.empty((B * K, D), dtype=np.float32)
    for c in range(N_CORES):
        out_full[row_idx[c]] = flat[row_idx[c]] + res.results[c]["out"]
    return out_full.reshape(B, K, D)


# revision 21
# speedup vs baseline: 1.0507x; 1.0131x over previous
"""CSWM transition GNN kernel for 8 TRN2 NeuronCores.

Sharding: data-parallel over the 512 edge-groups (the quirky edge list is
block-diagonal over groups of 15 consecutive flat rows). Each core gets
64 groups (960 edge rows) + 64 of the 512 zero-agg tail rows = 1024 node
rows. No cross-core communication.

Host-side algebra:
  - cat(xi,xi,xj)@e_w0 = xi@(W0a+W0b) + xj@W0c          (per-node U,V)
  - final edge matmul commutes with scatter-add; W2 then folds into the
    node MLP first layer: nw0s = e_w2 @ n_w0[532:1556]
  - per-edge work: one 1024x1024 matmul + LayerNorm + relu
"""

import numpy as np
import ml_dtypes

import concourse.bass as bass
import concourse.mybir as mybir
import concourse.tile as tile
from concourse import bacc
from concourse.bass_utils import run_bass_kernel_spmd
from concourse.masks import make_identity

BF16 = mybir.dt.bfloat16
F32 = mybir.dt.float32
F8 = mybir.dt.float8e4
DR = mybir.MatmulPerfMode.DoubleRow
AF = mybir.ActivationFunctionType

P = 128
D = 512            # embedding dim
H = 1024           # hidden dim
A_DIM = 20         # action dim
B = 512            # batch
K = 16             # objects
NG = 512           # total edge groups (block-diag over 15-row groups)
N_CORES = 8
G_CORE = NG // N_CORES          # 64 groups per core
EDGE_ROWS = G_CORE * 15         # 960
EXTRA_ROWS = (B * K - NG * 15) // N_CORES   # 64 zero-agg tail rows per core
N_ROWS = EDGE_ROWS + EXTRA_ROWS  # 1024 node rows per core
GB = 8                          # groups per aggregation block
NBLK = G_CORE // GB             # 8 blocks per core
E_BLK = GB * 225                # 1800 edges per block (incl. diagonal)
NCHUNK = (E_BLK + P - 1) // P   # 15 chunks of 128 edge-slots
NODES_BLK = GB * 15             # 120
E_PAD = 1808                    # E_BLK padded so fp8 DoubleRow k-pair stride %16==0
EPS = 1e-5
FP8 = True                      # fp8e4m3 DoubleRow for the edge matmul + aggregation


def _bf16(x):
    return np.ascontiguousarray(np.asarray(x, dtype=np.float32).astype(ml_dtypes.bfloat16))


def _f8(x):
    return np.ascontiguousarray(np.asarray(x, dtype=np.float32).astype(ml_dtypes.float8_e4m3))


def _f32(x):
    return np.ascontiguousarray(np.asarray(x, dtype=np.float32))


def _build_amat():
    """[NCHUNK*128, 120] 0/1 matrix: edge (gb,i,j) -> node gb*15+i, diagonal
    (j==i) excluded, padding rows zero."""
    a = np.zeros((NCHUNK * P, NODES_BLK), dtype=np.float32)
    for gb in range(GB):
        for i in range(15):
            for j in range(15):
                if i != j:
                    a[gb * 225 + i * 15 + j, gb * 15 + i] = 1.0
    return a


def _build_program(trivial_affine_e: bool, trivial_affine_n: bool):
    nc = bacc.Bacc("TRN2", target_bir_lowering=False, debug=False)

    # ---- DRAM parameters (per-core shards / replicated weights) ----
    def din(name, shape, dt):
        return nc.declare_dram_parameter(name, list(shape), dt, isOutput=False)

    xT = din("xT", (4, P, N_ROWS), BF16)       # x transposed, [ks,p,rows]
    actT = din("actT", (A_DIM + 1, N_ROWS), BF16)   # one-hot actions + edge-row indicator
    wab = din("wab", (4, P, H), BF16)          # W0a+W0b  [ks,p,out]
    w0c = din("w0c", (4, P, H), BF16)
    b0 = din("b0", (H,), F32)
    if FP8:
        w1 = din("w1", (8, P, H), F8)
        b1 = din("b1", (1, H), F8)
        amat = din("amat", (NCHUNK, P, P), F8)
    else:
        w1 = din("w1", (8, P, H), BF16)
        b1 = din("b1", (H,), F32)
        amat = din("amat", (NCHUNK, P, NODES_BLK), BF16)
    nw0x = din("nw0x", (4, P, H), BF16)
    nw0a = din("nw0a", (A_DIM + 1, H), BF16)   # rows 0..19 action, row 20 = e_b2 @ n_w0s
    nw0s = din("nw0s", (8, P, H), BF16)
    nb0 = din("nb0", (H,), F32)
    nw1 = din("nw1", (8, P, H), BF16)
    nb1 = din("nb1", (H,), F32)
    nw2 = din("nw2", (8, P, D), BF16)
    nb2 = din("nb2", (1, D), BF16)
    if not trivial_affine_e:
        e_g = din("e_g", (H,), F32)
        e_be = din("e_be", (H,), F32)
    if not trivial_affine_n:
        n_g = din("n_g", (H,), F32)
        n_be = din("n_be", (H,), F32)

    out = nc.declare_dram_parameter("out", [N_ROWS, D], F32, isOutput=True)

    with tile.TileContext(nc) as tc:
        with tc.tile_pool(name="const", bufs=1) as cpool:
            xT_s = cpool.tile([P, 4, N_ROWS], BF16)
            actT_s = cpool.tile([A_DIM + 1, N_ROWS], BF16)
            nc.sync.dma_start(actT_s[:], actT[:])
            ident = cpool.tile([P, P], BF16)
            make_identity(nc, ident)
            ones_row = cpool.tile([1, P], BF16)
            nc.vector.memset(ones_row[:], 1.0)
            eps_t = cpool.tile([P, 1], F32)
            nc.vector.memset(eps_t[:], EPS)
            # sT: aggregated-hidden, transposed [feat, rows]; tail rows zero
            sT = cpool.tile([P, 8, N_ROWS], BF16)
            nc.vector.memset(sT[:, :, EDGE_ROWS:N_ROWS], 0.0)

            # ================= EDGE PHASE =================
            with (
                tc.tile_pool(name="ew", bufs=1) as ew,
                tc.tile_pool(name="uv", bufs=1) as uvp,
                tc.tile_pool(name="rp", bufs=2) as rp,
                tc.tile_pool(name="zp", bufs=6) as zp,
                tc.tile_pool(name="st", bufs=2) as stp,
                tc.tile_pool(name="ps", bufs=3 if FP8 else 2, space="PSUM") as ps,
                tc.tile_pool(name="pa", bufs=1 if FP8 else 2, space="PSUM") as pa,
            ):
                wab_s = ew.tile([P, 4, H], BF16)
                w0c_s = ew.tile([P, 4, H], BF16)
                b0_t = ew.tile([P, 8], F32)
                nc.sync.dma_start(b0_t[:], b0[:].rearrange("(o p) -> p o", p=P))
                for ks in range(4):
                    nc.sync.dma_start(wab_s[:, ks, :], wab[ks])
                    nc.sync.dma_start(xT_s[:, ks, :], xT[ks])
                for ks in range(4):
                    nc.sync.dma_start(w0c_s[:, ks, :], w0c[ks])
                if FP8:
                    w1_s = ew.tile([P, 8, H], F8)
                    nc.gpsimd.dma_start(w1_s[:], w1[:].rearrange("k p n -> p k n"))
                    amat_s = ew.tile([P, NCHUNK, P], F8)
                    nc.gpsimd.dma_start(amat_s[:], amat[:].rearrange("c p n -> p c n"))
                    b1_r = ew.tile([1, H], F8)
                    nc.sync.dma_start(b1_r[:], b1[:])
                    ones8 = ew.tile([1, P], F8)
                    nc.vector.memset(ones8[:], 1.0)
                else:
                    w1_s = ew.tile([P, 8, H], BF16)
                    nc.sync.dma_start(w1_s[:], w1[:].rearrange("k p n -> p k n"))
                    amat_s = ew.tile([P, NCHUNK, NODES_BLK], BF16)
                    nc.sync.dma_start(amat_s[:], amat[:].rearrange("c p n -> p c n"))
                    b1_b = ew.tile([P, H], F32)
                    nc.sync.dma_start(b1_b[:], b1[None, :].to_broadcast((P, H)))
                if not trivial_affine_e:
                    eg_b = ew.tile([P, H], F32)
                    nc.sync.dma_start(eg_b[:], e_g[None, :].to_broadcast((P, H)))
                    ebe_b = ew.tile([P, H], F32)
                    nc.sync.dma_start(ebe_b[:], e_be[None, :].to_broadcast((P, H)))

                # ---- U = x@(W0a+W0b)+b0, V = x@W0c   (transposed layout) ----
                u_s = uvp.tile([P, 8, EDGE_ROWS], BF16, tag="u")
                v_s = uvp.tile([P, 8, EDGE_ROWS], BF16, tag="v")
                for m in range(8):
                    for dst, wt, bias in ((u_s, wab_s, True), (v_s, w0c_s, False)):
                        pt = ps.tile([P, H], F32, tag="mm")
                        for half, ncols in ((0, 512), (512, EDGE_ROWS - 512)):
                            for ks in range(4):
                                nc.tensor.matmul(
                                    pt[:, half:half + ncols],
                                    wt[:, ks, m * P:(m + 1) * P],
                                    xT_s[:, ks, half:half + ncols],
                                    start=(ks == 0), stop=(ks == 3),
                                )
                        nc.scalar.activation(
                            dst[:, m, :], pt[:, :EDGE_ROWS], AF.Identity,
                            bias=b0_t[:, m:m + 1] if bias else 0.0,
                        )

                # ---- per-block: build r, edge matmul + LN, aggregate ----
                s_blks = []

                def emit_agg(pagg, ch, z_t):
                    nc.tensor.matmul(pagg[:, 0:512], amat_s[:, ch, :NODES_BLK], z_t[:, 0:512],
                                     start=(ch == 0), stop=(ch == NCHUNK - 1))
                    nc.tensor.matmul(pagg[:, 512:1024], amat_s[:, ch, :NODES_BLK], z_t[:, 512:1024],
                                     start=(ch == 0), stop=(ch == NCHUNK - 1))

                def emit_agg_pair(pagg, cp, zpair):
                    # chunks (2cp, 2cp+1) in one DoubleRow matmul, K=256
                    lhs = amat_s[:, 2 * cp:2 * cp + 2, :NODES_BLK]
                    for half in (0, 512):
                        nc.tensor.matmul(pagg[:, half:half + 512], lhs,
                                         zpair[:, :, half:half + 512],
                                         start=(cp == 0), stop=False, perf_mode=DR)

                def emit_agg_last(pagg, zpair):
                    # chunk 14 alone (sub-slot 0 of the last pair tile), K=128
                    lhs = amat_s[:, NCHUNK - 1, :NODES_BLK]
                    for half in (0, 512):
                        nc.tensor.matmul(pagg[:, half:half + 512], lhs,
                                         zpair[:, 0, half:half + 512],
                                         start=False, stop=True)

                for blk in range(NBLK):
                    r_t = rp.tile([P, 8, E_PAD if FP8 else E_BLK], F8 if FP8 else BF16, tag="r")
                    col0 = blk * NODES_BLK
                    for fs in range(8):
                        u_sl = u_s[:, fs, col0:col0 + NODES_BLK]
                        v_sl = v_s[:, fs, col0:col0 + NODES_BLK]
                        u_in = u_sl.rearrange("p (g i) -> p g i", i=15)[:, :, :, None].to_broadcast((P, GB, 15, 15))
                        v_in = v_sl.rearrange("p (g j) -> p g j", j=15)[:, :, None, :].to_broadcast((P, GB, 15, 15))
                        if FP8:
                            rb = stp.tile([P, E_BLK], BF16, tag="rb")
                            rb_o = rb[:].rearrange("p (g i j) -> p g i j", i=15, j=15)
                            nc.vector.tensor_tensor(rb_o, u_in, v_in, mybir.AluOpType.add)
                            nc.scalar.activation(r_t[:, fs, 0:E_BLK], rb[:], AF.Relu)
                        else:
                            r_o = r_t[:, fs, :].rearrange("p (g i j) -> p g i j", i=15, j=15)
                            nc.vector.tensor_tensor(r_o, u_in, v_in, mybir.AluOpType.add)
                            nc.vector.tensor_scalar_max(r_t[:, fs, :], r_t[:, fs, :], 0.0)

                    pagg = pa.tile([NODES_BLK, H], F32, tag="agg")
                    z_tiles = []
                    for et in range(NCHUNK):
                        m_sz = min(P, E_BLK - et * P)
                        pt = ps.tile([P, H], F32, tag="mm")
                        if FP8:
                            for kp in range(4):
                                lhs = r_t[:, 2 * kp:2 * kp + 2, et * P:et * P + m_sz]
                                nc.tensor.matmul(pt[:m_sz, 0:512], lhs,
                                                 w1_s[:, 2 * kp:2 * kp + 2, 0:512],
                                                 start=(kp == 0), stop=False, perf_mode=DR)
                                nc.tensor.matmul(pt[:m_sz, 512:1024], lhs,
                                                 w1_s[:, 2 * kp:2 * kp + 2, 512:1024],
                                                 start=(kp == 0), stop=False, perf_mode=DR)
                            # bias b1 as a K=1 rank-1 update
                            nc.tensor.matmul(pt[:m_sz, 0:512], ones8[:, :m_sz], b1_r[:, 0:512],
                                             start=False, stop=True)
                            nc.tensor.matmul(pt[:m_sz, 512:1024], ones8[:, :m_sz], b1_r[:, 512:1024],
                                             start=False, stop=True)
                            h1b = pt
                        else:
                            for ks in range(8):
                                lhs = r_t[:, ks, et * P:et * P + m_sz]
                                nc.tensor.matmul(pt[:m_sz, 0:512], lhs, w1_s[:, ks, 0:512],
                                                 start=(ks == 0), stop=(ks == 7))
                                nc.tensor.matmul(pt[:m_sz, 512:1024], lhs, w1_s[:, ks, 512:1024],
                                                 start=(ks == 0), stop=(ks == 7))
                        if FP8:
                            if et % 2 == 0:
                                z_pair = zp.tile([P, 2, H], F8, tag="z")
                                z_tiles.append(z_pair)
                                if m_sz < P:
                                    nc.vector.memset(z_pair[:, 0, :], 0.0)
                            z_t = z_tiles[et // 2][:, et % 2, :]
                        else:
                            z_t = zp.tile([P, H], BF16, tag="z")
                            z_tiles.append(z_t)
                            if m_sz < P:
                                nc.vector.memset(z_t[:], 0.0)
                        # LayerNorm(h1 + b1) then relu; stats read PSUM directly
                        if not FP8:
                            h1b = stp.tile([P, H], F32, tag="h1b")
                            nc.vector.tensor_tensor(h1b[:m_sz], pt[:m_sz], b1_b[:m_sz], mybir.AluOpType.add)
                        st6 = stp.tile([P, 12], F32, tag="st6")
                        nc.vector.bn_stats(st6[:m_sz, 0:6], h1b[:m_sz, 0:512])
                        nc.vector.bn_stats(st6[:m_sz, 6:12], h1b[:m_sz, 512:1024])
                        mv = stp.tile([P, 2], F32, tag="mv")
                        nc.vector.bn_aggr(mv[:m_sz], st6[:m_sz].rearrange("p (a b) -> p a b", b=6))
                        sc = stp.tile([P, 2], F32, tag="sc")
                        nc.scalar.activation(sc[:m_sz, 0:1], mv[:m_sz, 1:2],
                                             AF.Abs_reciprocal_sqrt, bias=eps_t[:m_sz])
                        nc.vector.tensor_scalar(sc[:m_sz, 1:2], mv[:m_sz, 0:1],
                                                sc[:m_sz, 0:1], -1.0,
                                                mybir.AluOpType.mult, mybir.AluOpType.mult)
                        if trivial_affine_e:
                            nc.scalar.activation(z_t[:m_sz], h1b[:m_sz], AF.Relu,
                                                 bias=sc[:m_sz, 1:2], scale=sc[:m_sz, 0:1])
                        else:
                            zn = stp.tile([P, H], F32, tag="zn")
                            nc.scalar.activation(zn[:m_sz], h1b[:m_sz], AF.Identity,
                                                 bias=sc[:m_sz, 1:2], scale=sc[:m_sz, 0:1])
                            nc.vector.tensor_tensor(zn[:m_sz], zn[:m_sz], eg_b[:m_sz], mybir.AluOpType.mult)
                            nc.vector.tensor_tensor(zn[:m_sz], zn[:m_sz], ebe_b[:m_sz], mybir.AluOpType.add)
                            nc.scalar.activation(z_t[:m_sz], zn[:m_sz], AF.Relu)
                        # interleave aggregation, trailing the LN pipeline
                        if FP8:
                            if et % 2 == 1 and et >= 3:
                                emit_agg_pair(pagg, (et - 3) // 2, z_tiles[(et - 3) // 2])
                        else:
                            if et >= 2:
                                emit_agg(pagg, et - 2, z_tiles[et - 2])
                    if FP8:
                        emit_agg_pair(pagg, 6, z_tiles[6])
                        emit_agg_last(pagg, z_tiles[7])
                    else:
                        emit_agg(pagg, NCHUNK - 2, z_tiles[NCHUNK - 2])
                        emit_agg(pagg, NCHUNK - 1, z_tiles[NCHUNK - 1])

                    # evict aggregated block (transposed into sT at node-phase start)
                    s_blk = cpool.tile([P, H], BF16, tag=f"sblk{blk}")
                    s_blks.append(s_blk)
                    nc.scalar.activation(s_blk[0:NODES_BLK, :], pagg[:], AF.Identity)

            # ================= NODE PHASE =================
            with (
                tc.tile_pool(name="nw", bufs=1) as nw,
                tc.tile_pool(name="nact", bufs=1) as na,
                tc.tile_pool(name="nst", bufs=3) as nst,
                tc.tile_pool(name="ps2", bufs=2, space="PSUM") as ps2,
                tc.tile_pool(name="pa2", bufs=2, space="PSUM") as pa2,
            ):
                nw0x_s = nw.tile([P, 4, H], BF16)
                nc.gpsimd.dma_start(nw0x_s[:], nw0x[:].rearrange("k p n -> p k n"))
                nw0a_s = nw.tile([A_DIM + 1, H], BF16)
                nc.sync.dma_start(nw0a_s[:], nw0a[:])
                nw0s_s = nw.tile([P, 8, H], BF16)
                nc.gpsimd.dma_start(nw0s_s[:], nw0s[:].rearrange("k p n -> p k n"))
                nw1_s = nw.tile([P, 8, H], BF16)
                nc.gpsimd.dma_start(nw1_s[:], nw1[:].rearrange("k p n -> p k n"))
                nw2_s = nw.tile([P, 8, D], BF16)
                nc.gpsimd.dma_start(nw2_s[:], nw2[:].rearrange("k p n -> p k n"))
                nb0_t = nw.tile([P, 8], F32)
                nc.sync.dma_start(nb0_t[:], nb0[:].rearrange("(o p) -> p o", p=P))
                nb1_b = nw.tile([P, H], F32)
                nc.sync.dma_start(nb1_b[:], nb1[None, :].to_broadcast((P, H)))
                nb2_s = nw.tile([1, D], BF16)
                nc.sync.dma_start(nb2_s[:], nb2[:])
                if not trivial_affine_n:
                    ng_b = nw.tile([P, H], F32)
                    nc.sync.dma_start(ng_b[:], n_g[None, :].to_broadcast((P, H)))
                    nbe_b = nw.tile([P, H], F32)
                    nc.sync.dma_start(nbe_b[:], n_be[None, :].to_broadcast((P, H)))

                # ---- transpose aggregated blocks into sT ----
                for blk in range(NBLK):
                    for fs in range(8):
                        ptp = pa2.tile([P, P], BF16, tag="tp")
                        nc.tensor.transpose(
                            ptp[:, 0:NODES_BLK],
                            s_blks[blk][0:NODES_BLK, fs * P:(fs + 1) * P],
                            ident[0:NODES_BLK, 0:NODES_BLK],
                        )
                        nc.scalar.activation(
                            sT[:, fs, blk * NODES_BLK:(blk + 1) * NODES_BLK],
                            ptp[:, 0:NODES_BLK], AF.Identity)

                # ---- node layer 1 -> hT (transposed out, relu+bias in evict) ----
                hT = na.tile([P, 8, N_ROWS], BF16, tag="hT")
                for m in range(8):
                    pt = ps2.tile([P, H], F32, tag="mm")
                    msl = slice(m * P, (m + 1) * P)
                    for half in (0, 512):
                        sl = slice(half, half + 512)
                        chunks = (
                            [(nw0x_s[:, ks, msl], xT_s[:, ks, sl]) for ks in range(4)]
                            + [(nw0a_s[:, msl], actT_s[:, sl])]
                            + [(nw0s_s[:, ks, msl], sT[:, ks, sl]) for ks in range(8)]
                        )
                        for ci, (lhs, rhs) in enumerate(chunks):
                            nc.tensor.matmul(pt[:, sl], lhs, rhs,
                                             start=(ci == 0), stop=(ci == len(chunks) - 1))
                    nc.scalar.activation(hT[:, m, :], pt[:], AF.Relu, bias=nb0_t[:, m:m + 1])

                # ---- node layer 2 (row-major out) + LN + relu -> z2, transpose, layer 3 ----
                z2T = na.tile([P, 8, N_ROWS], BF16, tag="z2T")
                for rt in range(8):
                    pt = ps2.tile([P, H], F32, tag="mm")
                    for ks in range(8):
                        lhs = hT[:, ks, rt * P:(rt + 1) * P]
                        nc.tensor.matmul(pt[:, 0:512], lhs, nw1_s[:, ks, 0:512],
                                         start=(ks == 0), stop=(ks == 7))
                        nc.tensor.matmul(pt[:, 512:1024], lhs, nw1_s[:, ks, 512:1024],
                                         start=(ks == 0), stop=(ks == 7))
                    h2b = nst.tile([P, H], F32, tag="h2b")
                    nc.vector.tensor_tensor(h2b[:], pt[:], nb1_b[:], mybir.AluOpType.add)
                    st6 = nst.tile([P, 12], F32, tag="st6")
                    nc.vector.bn_stats(st6[:, 0:6], h2b[:, 0:512])
                    nc.vector.bn_stats(st6[:, 6:12], h2b[:, 512:1024])
                    mv = nst.tile([P, 2], F32, tag="mv")
                    nc.vector.bn_aggr(mv[:], st6[:].rearrange("p (a b) -> p a b", b=6))
                    sc = nst.tile([P, 2], F32, tag="sc")
                    nc.scalar.activation(sc[:, 0:1], mv[:, 1:2],
                                         AF.Abs_reciprocal_sqrt, bias=eps_t[:])
                    nc.vector.tensor_scalar(sc[:, 1:2], mv[:, 0:1], sc[:, 0:1], -1.0,
                                            mybir.AluOpType.mult, mybir.AluOpType.mult)
                    z2 = nst.tile([P, H], BF16, tag="z2")
                    if trivial_affine_n:
                        nc.scalar.activation(z2[:], h2b[:], AF.Relu,
                                             bias=sc[:, 1:2], scale=sc[:, 0:1])
                    else:
                        zn = nst.tile([P, H], F32, tag="zn")
                        nc.scalar.activation(zn[:], h2b[:], AF.Identity,
                                             bias=sc[:, 1:2], scale=sc[:, 0:1])
                        nc.vector.tensor_tensor(zn[:], zn[:], ng_b[:], mybir.AluOpType.mult)
                        nc.vector.tensor_tensor(zn[:], zn[:], nbe_b[:], mybir.AluOpType.add)
                        nc.scalar.activation(z2[:], zn[:], AF.Relu)
                    for fs in range(8):
                        ptp = pa2.tile([P, P], BF16, tag="tp")
                        nc.tensor.transpose(ptp[:], z2[:, fs * P:(fs + 1) * P], ident[:])
                        nc.scalar.activation(z2T[:, fs, rt * P:(rt + 1) * P], ptp[:], AF.Identity)

                # ---- node layer 3 + bias ----
                out_r = out[:].rearrange("(rt p) d -> p rt d", p=P)
                for rt in range(8):
                    pt = ps2.tile([P, H], F32, tag="mm")
                    for ks in range(8):
                        nc.tensor.matmul(pt[:, 0:D], z2T[:, ks, rt * P:(rt + 1) * P],
                                         nw2_s[:, ks, :], start=(ks == 0), stop=False)
                    nc.tensor.matmul(pt[:, 0:D], ones_row[:], nb2_s[:], start=False, stop=True)
                    outb = nst.tile([P, D], F32, tag="outb")
                    nc.scalar.activation(outb[:], pt[:, 0:D], AF.Identity)
                    nc.sync.dma_start(out_r[:, rt, :], outb[:])

    return nc


_PROG_CACHE = {}


def _get_program(trivial_e, trivial_n):
    key = (trivial_e, trivial_n, FP8)
    if key not in _PROG_CACHE:
        nc = _build_program(trivial_e, trivial_n)
        nc.finalize()
        _PROG_CACHE[key] = nc
    return _PROG_CACHE[key]


def kernel(states, action, e_w0, e_b0, e_w1, e_b1, e_g, e_be, e_w2, e_b2,
           n_w0, n_b0, n_w1, n_b1, n_g, n_be, n_w2, n_b2):
    states = _f32(states)
    action = np.asarray(action).astype(np.int64)
    e_w0, e_b0, e_w1, e_b1 = _f32(e_w0), _f32(e_b0), _f32(e_w1), _f32(e_b1)
    e_g, e_be, e_w2, e_b2 = _f32(e_g), _f32(e_be), _f32(e_w2), _f32(e_b2)
    n_w0, n_b0, n_w1, n_b1 = _f32(n_w0), _f32(n_b0), _f32(n_w1), _f32(n_b1)
    n_g, n_be, n_w2, n_b2 = _f32(n_g), _f32(n_be), _f32(n_w2), _f32(n_b2)

    trivial_e = bool(np.all(e_g == 1.0) and np.all(e_be == 0.0))
    trivial_n = bool(np.all(n_g == 1.0) and np.all(n_be == 0.0))
    nc = _get_program(trivial_e, trivial_n)

    flat = states.reshape(-1, D)                        # [8192, 512]
    # one-hot action vectors per flat row
    av = np.zeros((B, A_DIM * K), dtype=np.float32)
    av[np.arange(B), action] = 1.0
    av = av.reshape(-1, A_DIM)                          # [8192, 20]

    # host-folded weights
    wab = e_w0[0:D] + e_w0[D:2 * D]                     # [512, 1024]
    w0c = e_w0[2 * D:3 * D]
    nw0x = n_w0[0:D]
    nw0a = n_w0[D:D + A_DIM]
    n_w0s_part = n_w0[D + A_DIM:]
    nw0s = e_w2 @ n_w0s_part                            # [1024, 1024]
    nb0 = n_b0
    nw0a21 = np.concatenate([nw0a, (e_b2 @ n_w0s_part).reshape(1, H)], axis=0)

    amat = _build_amat()

    def kslice(w, kt):   # [K, N] -> [K/128, 128, N]
        return w.reshape(kt, P, w.shape[1])

    if FP8:
        amat_in = _f8(np.concatenate(
            [amat.reshape(NCHUNK, P, NODES_BLK),
             np.zeros((NCHUNK, P, P - NODES_BLK), np.float32)], axis=2))
        w1_in = _f8(kslice(e_w1, 8))
        b1_in = _f8(e_b1.reshape(1, H))
    else:
        amat_in = _bf16(amat.reshape(NCHUNK, P, NODES_BLK))
        w1_in = _bf16(kslice(e_w1, 8))
        b1_in = _f32(e_b1)
    common = {
        "wab": _bf16(kslice(wab, 4)), "w0c": _bf16(kslice(w0c, 4)),
        "b0": _f32(e_b0), "w1": w1_in, "b1": b1_in,
        "amat": amat_in,
        "nw0x": _bf16(kslice(nw0x, 4)), "nw0a": _bf16(nw0a21),
        "nw0s": _bf16(kslice(nw0s, 8)), "nb0": _f32(nb0),
        "nw1": _bf16(kslice(n_w1, 8)), "nb1": _f32(n_b1),
        "nw2": _bf16(kslice(n_w2, 8)), "nb2": _bf16(n_b2.reshape(1, D)),
    }
    if not trivial_e:
        common["e_g"] = _f32(e_g)
        common["e_be"] = _f32(e_be)
    if not trivial_n:
        common["n_g"] = _f32(n_g)
        common["n_be"] = _f32(n_be)

    in_maps = []
    row_idx = []
    for c in range(N_CORES):
        idx = np.concatenate([
            np.arange(c * EDGE_ROWS, (c + 1) * EDGE_ROWS),
            np.arange(NG * 15 + c * EXTRA_ROWS, NG * 15 + (c + 1) * EXTRA_ROWS),
        ])
        row_idx.append(idx)
        x_rows = flat[idx]                              # [1024, 512]
        xt = np.ascontiguousarray(x_rows.T)             # [512, 1024]
        at = np.concatenate([av[idx].T, np.concatenate(
            [np.full((1, EDGE_ROWS), 14.0, np.float32),
             np.zeros((1, EXTRA_ROWS), np.float32)], axis=1)], axis=0)  # [21, 1024]
        m = dict(common)
        m["xT"] = _bf16(xt.reshape(4, P, N_ROWS))
        m["actT"] = _bf16(at)
        in_maps.append(m)

    res = run_bass_kernel_spmd(nc, in_maps, core_ids=list(range(N_CORES)))
    global LAST_RESULT
    LAST_RESULT = res

    out_full = np.empty((B * K, D), dtype=np.float32)
    for c in range(N_CORES):
        out_full[row_idx[c]] = flat[row_idx[c]] + res.results[c]["out"]
    return out_full.reshape(B, K, D)
